# revision 1
# baseline (speedup 1.0000x reference)
"""Trainium2 Bass kernel for nn_ADFCell — scan-Picard block algorithm.

Per block of T=64 steps the coupled (w, f) recurrence is solved by K=10
fixed-point iterations; each iteration is bulk DVE work + two
tensor_tensor_scan ops (the f linear recurrence given v) + a PE Gram
mat-vec (overlapped with the DVE work). Gradient-clip events enter as
scan-input patches refreshed at iterations {1,4,7,9}. Converges to the
exact fp32 trajectory (numpy-validated rel err 5.9e-4).

Data parallel: 8 examples/core on 8 cores; partitions p = 2b+i.
"""

import numpy as np

import concourse.bacc as bacc
import concourse.bass as bass
import concourse.mybir as mybir
from concourse import masks
from concourse.bass_utils import run_bass_kernel_spmd
from concourse.tile import TileContext

Alu = mybir.AluOpType
f32 = mybir.dt.float32

B, L, TAPS, NM = 64, 4096, 32, 2
NCORES = 8
BC = B // NCORES            # 8 examples per core
P = 16                      # partitions p = 2b+i
KD = 2 * TAPS * NM          # 128 real k-dim ([ur|ui] stacking)
T = 64                      # block length
T2 = 2 * T
KIT = 10                    # Picard iterations
CLIP_SET = (1, 4, 7, 9)
LR_W = 1.0 / 2**6
LR_F = 1.0 / 2**7
GMAX = 30.0
EPS = 1e-9

_BUILT = {}


def _s2(ap):
    """[p, (2 c)] -> [p, 2, c] split view."""
    return ap.rearrange("p (s c) -> p s c", s=2)


def _build_nc(nblk, static=False):
    nc = bacc.Bacc("TRN2", target_bir_lowering=False, debug=False)
    rhsu = nc.declare_dram_parameter("a_rhsu", [nblk, BC, KD, T2], f32, isOutput=False)
    glhs = nc.declare_dram_parameter("a_glhs", [nblk, BC, KD, T], f32, isOutput=False)
    u4d = nc.declare_dram_parameter("a_u4", [nblk, BC, T2, KD], f32, isOutput=False)
    xpd = nc.declare_dram_parameter("a_xp", [nblk, P, 2, T2], f32, isOutput=False)
    w0t = nc.declare_dram_parameter("a_w0t", [KD, P], f32, isOutput=False)
    f0t = nc.declare_dram_parameter("a_f0t", [P, 2], f32, isOutput=False)
    phi0t = nc.declare_dram_parameter("a_phi0t", [P, 2], f32, isOutput=False)
    trid = nc.declare_dram_parameter("a_tri", [T, T2], f32, isOutput=False)
    trisd = nc.declare_dram_parameter("a_trisgn", [T, T2], f32, isOutput=False)
    kod = nc.declare_dram_parameter("ko", [P, nblk, T2], f32, isOutput=True)

    shuf_mask = [p ^ 1 for p in range(P)] + [0] * 16

    with TileContext(nc) as tc:
        with (
            tc.tile_pool(name="st", bufs=1) as st,
            tc.tile_pool(name="io", bufs=2) as io,
            tc.tile_pool(name="gvp", bufs=2) as gvp,
            tc.tile_pool(name="itp", bufs=1) as itp,
            tc.tile_pool(name="scp", bufs=2) as scp,
            tc.tile_pool(name="psv", bufs=2, space="PSUM") as psv,
            tc.tile_pool(name="psa", bufs=2, space="PSUM") as psa,
            tc.tile_pool(name="psb", bufs=1, space="PSUM") as psb,
        ):
            WT = st.tile([KD, P], f32)
            F = st.tile([P, 2 * (T + 1)], f32)
            PHI0 = st.tile([P, 2], f32)
            TRI = st.tile([T, T2], f32)
            TRISGN = st.tile([T, T2], f32)
            IDP = st.tile([P, P], f32)
            IDK = st.tile([KD, KD], f32)
            masks.make_identity(nc, IDP[:])
            masks.make_identity(nc, IDK[:])
            nc.sync.dma_start(out=WT[:], in_=w0t[:])
            nc.sync.dma_start(out=F[:, 0:1], in_=f0t[:, 0:1])
            nc.sync.dma_start(out=F[:, T + 1:T + 2], in_=f0t[:, 1:2])
            nc.sync.dma_start(out=PHI0[:], in_=phi0t[:])
            nc.sync.dma_start(out=TRI[:], in_=trid[:])
            nc.sync.dma_start(out=TRISGN[:], in_=trisd[:])

            from contextlib import nullcontext
            loop_cm = nullcontext(0) if (static and nblk == 1) else tc.For_i(0, nblk, 1)
            with loop_cm as mI:
                # ---- DMA block inputs ----
                RH = io.tile([KD, BC * T2], f32)
                GL = io.tile([KD, BC * T], f32)
                U4 = io.tile([T2, BC * KD], f32)
                XP = io.tile([P, 2 * T2], f32)
                nc.sync.dma_start(
                    out=RH[:].rearrange("k (b c) -> k b c", b=BC),
                    in_=rhsu[bass.ds(mI, 1), :, :, :].rearrange("o b k c -> k (o b) c"))
                nc.sync.dma_start(
                    out=GL[:].rearrange("k (b c) -> k b c", b=BC),
                    in_=glhs[bass.ds(mI, 1), :, :, :].rearrange("o b k c -> k (o b) c"))
                nc.gpsimd.dma_start(
                    out=U4[:].rearrange("k (b c) -> k b c", b=BC),
                    in_=u4d[bass.ds(mI, 1), :, :, :].rearrange("o b k c -> k (o b) c"))
                nc.gpsimd.dma_start(
                    out=XP[:].rearrange("p (a c) -> p a c", a=2),
                    in_=xpd[bass.ds(mI, 1), :, :, :].rearrange("o p a c -> (o p) a c"))
                XPa = _s2(XP[:, 0:T2])
                XPb = _s2(XP[:, T2:2 * T2])

                # ---- vtilde (needs WT from prev block: serial entry) ----
                PVTT = psb.tile([T2, P], f32, tag="tsp")
                for b in range(BC):
                    nc.tensor.matmul(out=PVTT[:, 2 * b:2 * b + 2],
                                     lhsT=RH[:, b * T2:(b + 1) * T2],
                                     rhs=WT[:, 2 * b:2 * b + 2],
                                     start=True, stop=True)
                VTT = itp.tile([T2, P], f32, tag="vtt")
                nc.scalar.copy(out=VTT[:], in_=PVTT[:])
                PVT = psv.tile([P, T2], f32, tag="pv")
                nc.tensor.transpose(PVT[:], VTT[:], IDK[:])
                VT = itp.tile([P, T2], f32, tag="vt")
                nc.scalar.copy(out=VT[:], in_=PVT[:])

                # ---- Gram + GV build (PE + DVE/Pool, overlaps iter 0) ----
                GV = gvp.tile([T2, BC * T2], f32)
                for b in range(BC):
                    PG = psa.tile([T, T2], f32, tag="pg")
                    nc.tensor.matmul(out=PG[:], lhsT=GL[:, b * T:(b + 1) * T],
                                     rhs=RH[:, b * T2:(b + 1) * T2],
                                     start=True, stop=True)
                    nc.vector.tensor_tensor(
                        out=GV[0:T, b * T2:(b + 1) * T2], in0=PG[:], in1=TRI[:],
                        op=Alu.mult)
                    gsw = GV[0:T, b * T2:(b + 1) * T2].rearrange(
                        "r (h c) -> r h c", h=2)[:, ::-1]
                    nc.gpsimd.tensor_tensor(
                        out=GV[T:T2, b * T2:(b + 1) * T2].rearrange("r (h c) -> r h c", h=2),
                        in0=gsw, in1=TRISGN[:].rearrange("r (h c) -> r h c", h=2),
                        op=Alu.mult)

                # ---- iteration tiles ----
                E = itp.tile([P, T2], f32, tag="e")
                ETS = itp.tile([T2, P], f32, tag="ets")
                PHI = itp.tile([P, T2], f32, tag="phi")
                Mc = itp.tile([P, T], f32, tag="mc")
                FM = itp.tile([P, T2], f32, tag="fm")
                Vs = itp.tile([P, T2], f32, tag="vs")

                fprev = _s2(F[:])[:, :, 0:T]
                ftail = _s2(F[:])[:, :, T:T + 1]

                for k in range(KIT):
                    if k == 0:
                        v2, v3 = VT[:], _s2(VT[:])
                        phiv = PHI0[:].broadcast_to((P, 2, T))
                    else:
                        PET = psb.tile([T2, P], f32, tag="tsp")
                        nc.tensor.transpose(PET[:], E[:], IDP[:])
                        nc.scalar.copy(out=ETS[:], in_=PET[:])
                        PLV = psb.tile([T2, P], f32, tag="tsp")
                        for b in range(BC):
                            nc.tensor.matmul(out=PLV[:, 2 * b:2 * b + 2],
                                             lhsT=GV[:, b * T2:(b + 1) * T2],
                                             rhs=ETS[:, 2 * b:2 * b + 2],
                                             start=True, stop=True)
                        LVS = itp.tile([T2, P], f32, tag="lvs")
                        nc.scalar.copy(out=LVS[:], in_=PLV[:])
                        PV = psv.tile([P, T2], f32, tag="pv")
                        nc.tensor.transpose(PV[:], LVS[:], IDK[:])
                        nc.vector.tensor_tensor(out=Vs[:], in0=VT[:], in1=PV[:],
                                                op=Alu.add)
                        v2, v3 = Vs[:], _s2(Vs[:])
                        phiv = _s2(PHI[:])

                    # e = x*psi - v
                    AB = scp.tile([P, 4 * T], f32, tag="ab")
                    nc.vector.tensor_tensor(out=_s2(AB[:, 0:T2]), in0=XPa, in1=phiv,
                                            op=Alu.mult)
                    nc.vector.tensor_tensor(out=_s2(AB[:, T2:4 * T]), in0=XPb, in1=phiv,
                                            op=Alu.mult)
                    ab4 = AB[:].rearrange("p (a h c) -> p a h c", a=2, h=2)
                    XPS = scp.tile([P, T2], f32, tag="xps")
                    nc.vector.tensor_tensor(out=_s2(XPS[:]), in0=ab4[:, :, 1],
                                            in1=ab4[:, :, 0], op=Alu.add)
                    nc.vector.tensor_tensor(out=E[:], in0=XPS[:], in1=v2, op=Alu.subtract)

                    # f-side
                    VV = scp.tile([P, T2], f32, tag="vv")
                    nc.vector.tensor_tensor(out=VV[:], in0=v2, in1=v2, op=Alu.mult)
                    M2 = scp.tile([32, T], f32, tag="m2")
                    nc.vector.tensor_tensor(out=M2[0:P, :], in0=VV[:, 0:T],
                                            in1=VV[:, T:T2], op=Alu.add)
                    VES = scp.tile([32, T], f32, tag="ves")
                    nc.vector.stream_shuffle(out=VES[:], in_=M2[:], mask=shuf_mask)
                    VE = scp.tile([P, T], f32, tag="ve")
                    nc.vector.scalar_tensor_tensor(out=VE[:], in0=VES[0:P, :], scalar=EPS,
                                                   in1=M2[0:P, :], op0=Alu.add, op1=Alu.add)
                    RV = scp.tile([P, T], f32, tag="rv")
                    nc.vector.reciprocal_approx_fast(out=RV[:], in_=VE[:])
                    GT = scp.tile([P, T], f32, tag="gt")
                    nc.vector.scalar_tensor_tensor(out=GT[:], in0=M2[0:P, :], scalar=-LR_F,
                                                   in1=RV[:], op0=Alu.mult, op1=Alu.mult)
                    AT = scp.tile([P, T], f32, tag="at")
                    nc.vector.tensor_scalar_add(out=AT[:], in0=GT[:], scalar1=1.0)
                    CD = scp.tile([P, 4 * T], f32, tag="cd")
                    nc.gpsimd.tensor_tensor(out=_s2(CD[:, 0:T2]), in0=XPa, in1=v3,
                                            op=Alu.mult)
                    nc.gpsimd.tensor_tensor(out=_s2(CD[:, T2:4 * T]), in0=XPb, in1=v3,
                                            op=Alu.mult)
                    cd4 = CD[:].rearrange("p (a h c) -> p a h c", a=2, h=2)
                    XCV = scp.tile([P, T2], f32, tag="xcv")
                    nc.gpsimd.tensor_tensor(out=_s2(XCV[:]), in0=cd4[:, :, 1],
                                            in1=cd4[:, :, 0], op=Alu.add)
                    BT = scp.tile([P, T2], f32, tag="bt")
                    nc.vector.scalar_tensor_tensor(out=_s2(BT[:]), in0=_s2(XCV[:]),
                                                   scalar=LR_F,
                                                   in1=RV[:].broadcast_to((P, T, 2)).rearrange("p c s -> p s c"),
                                                   op0=Alu.mult, op1=Alu.mult)

                    if k < 2:
                        a2, b2 = AT[:], _s2(BT[:])
                    else:
                        A2 = scp.tile([P, T], f32, tag="a2")
                        B2 = scp.tile([P, T2], f32, tag="b2")
                        nc.vector.tensor_tensor(out=A2[:], in0=AT[:], in1=Mc[:],
                                                op=Alu.mult)
                        nc.vector.tensor_tensor(out=_s2(B2[:]), in0=_s2(BT[:]),
                                                in1=Mc[:].broadcast_to((P, T, 2)).rearrange("p c s -> p s c"),
                                                op=Alu.mult)
                        nc.vector.tensor_tensor(out=B2[:], in0=B2[:], in1=FM[:],
                                                op=Alu.add)
                        a2, b2 = A2[:], _s2(B2[:])
                    nc.vector.tensor_tensor_scan(
                        out=F[:, 1:T + 1], data0=a2, data1=b2[:, 0],
                        initial=F[:, 0:1], op0=Alu.mult, op1=Alu.add)
                    nc.vector.tensor_tensor_scan(
                        out=F[:, T + 2:2 * T + 2], data0=a2, data1=b2[:, 1],
                        initial=F[:, T + 1:T + 2], op0=Alu.mult, op1=Alu.add)

                    if k in CLIP_SET:
                        M2L = scp.tile([P, T], f32, tag="m2l")
                        nc.vector.tensor_scalar_mul(out=M2L[:], in0=M2[0:P, :],
                                                    scalar1=LR_F)
                        P1 = scp.tile([P, T2], f32, tag="p1")
                        nc.vector.tensor_tensor(out=_s2(P1[:]), in0=fprev,
                                                in1=M2L[:].broadcast_to((P, T, 2)).rearrange("p c s -> p s c"),
                                                op=Alu.mult)
                        NL = scp.tile([P, T2], f32, tag="nl")
                        nc.vector.scalar_tensor_tensor(out=NL[:], in0=XCV[:], scalar=LR_F,
                                                       in1=P1[:], op0=Alu.mult,
                                                       op1=Alu.subtract)
                        E2 = scp.tile([P, T2], f32, tag="e2")
                        nc.vector.tensor_tensor(out=E2[:], in0=NL[:], in1=NL[:],
                                                op=Alu.mult)
                        N2 = scp.tile([P, T], f32, tag="n2")
                        nc.vector.scalar_tensor_tensor(out=N2[:], in0=E2[:, 0:T],
                                                       scalar=1e-30, in1=E2[:, T:T2],
                                                       op0=Alu.add, op1=Alu.add)
                        VL2 = scp.tile([P, T], f32, tag="vl2")
                        nc.vector.tensor_tensor(out=VL2[:], in0=VE[:], in1=VE[:],
                                                op=Alu.mult)
                        MINV = scp.tile([P, T], f32, tag="minv")
                        nc.vector.tensor_scalar(out=MINV[:], in0=VL2[:],
                                                scalar1=(GMAX * LR_F) ** 2,
                                                scalar2=None, op0=Alu.mult)
                        nc.vector.tensor_tensor(out=MINV[:], in0=N2[:], in1=MINV[:],
                                                op=Alu.is_gt)
                        nc.vector.tensor_scalar(out=Mc[:], in0=MINV[:], scalar1=-1.0,
                                                scalar2=1.0, op0=Alu.mult, op1=Alu.add)
                        RS = scp.tile([P, T], f32, tag="rs")
                        nc.vector.reciprocal_approx_fast(out=RS[:], in_=N2[:])
                        SQ = scp.tile([P, T], f32, tag="sq")
                        nc.scalar.sqrt(out=SQ[:], in_=RS[:])
                        D4 = scp.tile([P, T2], f32, tag="d4")
                        nc.vector.tensor_tensor(out=_s2(D4[:]), in0=_s2(NL[:]),
                                                in1=SQ[:].broadcast_to((P, T, 2)).rearrange("p c s -> p s c"),
                                                op=Alu.mult)
                        FCL = scp.tile([P, T2], f32, tag="fcl")
                        nc.vector.scalar_tensor_tensor(out=_s2(FCL[:]), in0=_s2(D4[:]),
                                                       scalar=GMAX * LR_F, in1=fprev,
                                                       op0=Alu.mult, op1=Alu.add)
                        nc.vector.tensor_tensor(out=_s2(FM[:]), in0=_s2(FCL[:]),
                                                in1=MINV[:].broadcast_to((P, T, 2)).rearrange("p c s -> p s c"),
                                                op=Alu.mult)

                    if k < KIT - 1:
                        FF = scp.tile([P, T2], f32, tag="ff")
                        nc.vector.tensor_tensor(out=_s2(FF[:]), in0=fprev, in1=fprev,
                                                op=Alu.mult)
                        H2 = scp.tile([P, T], f32, tag="h2")
                        nc.vector.scalar_tensor_tensor(out=H2[:], in0=FF[:, 0:T],
                                                       scalar=1e-30, in1=FF[:, T:T2],
                                                       op0=Alu.add, op1=Alu.add)
                        RH2 = scp.tile([P, T], f32, tag="rh2")
                        nc.vector.reciprocal_approx_fast(out=RH2[:], in_=H2[:])
                        RF = scp.tile([P, T], f32, tag="rf")
                        nc.scalar.sqrt(out=RF[:], in_=RH2[:])
                        nc.vector.tensor_tensor(out=_s2(PHI[:]), in0=fprev,
                                                in1=RF[:].broadcast_to((P, T, 2)).rearrange("p c s -> p s c"),
                                                op=Alu.mult)

                # ---- outputs k = v * f ----
                KA = scp.tile([P, T2], f32, tag="ka")
                KB = scp.tile([P, T2], f32, tag="kb")
                nc.gpsimd.tensor_tensor(out=_s2(KA[:]), in0=_s2(Vs[:]), in1=fprev,
                                        op=Alu.mult)
                fsw = _s2(F[:])[:, ::-1, 0:T]
                nc.gpsimd.tensor_tensor(out=_s2(KB[:]), in0=_s2(Vs[:]), in1=fsw,
                                        op=Alu.mult)
                KO = scp.tile([P, T2], f32, tag="kot")
                nc.gpsimd.tensor_tensor(out=KO[:, 0:T], in0=KA[:, 0:T], in1=KA[:, T:T2],
                                        op=Alu.subtract)
                nc.gpsimd.tensor_tensor(out=KO[:, T:T2], in0=KB[:, 0:T], in1=KB[:, T:T2],
                                        op=Alu.add)
                nc.gpsimd.dma_start(
                    out=kod[:, bass.ds(mI, 1), :],
                    in_=KO[:].rearrange("p (o c) -> p o c", o=1))

                # ---- w update ----
                PET2 = psb.tile([T2, P], f32, tag="tsp")
                nc.tensor.transpose(PET2[:], E[:], IDP[:])
                nc.scalar.copy(out=ETS[:], in_=PET2[:])
                PDW = psb.tile([KD, P], f32, tag="tsp")
                for b in range(BC):
                    nc.tensor.matmul(out=PDW[:, 2 * b:2 * b + 2],
                                     lhsT=U4[:, b * KD:(b + 1) * KD],
                                     rhs=ETS[:, 2 * b:2 * b + 2],
                                     start=True, stop=True)
                nc.vector.tensor_tensor(out=WT[:], in0=WT[:], in1=PDW[:], op=Alu.add)

                # ---- f carry + phi0 ----
                FF0 = scp.tile([P, 2], f32, tag="ff0")
                nc.vector.tensor_tensor(out=FF0[:], in0=ftail.rearrange("p s c -> p (s c)"),
                                        in1=ftail.rearrange("p s c -> p (s c)"),
                                        op=Alu.mult)
                H20 = scp.tile([P, 1], f32, tag="h20")
                nc.vector.scalar_tensor_tensor(out=H20[:], in0=FF0[:, 0:1], scalar=1e-30,
                                               in1=FF0[:, 1:2], op0=Alu.add, op1=Alu.add)
                RH0 = scp.tile([P, 1], f32, tag="rh0")
                nc.vector.reciprocal_approx_fast(out=RH0[:], in_=H20[:])
                RF0 = scp.tile([P, 1], f32, tag="rf0")
                nc.scalar.sqrt(out=RF0[:], in_=RH0[:])
                nc.vector.tensor_tensor(out=PHI0[:],
                                        in0=ftail.rearrange("p s c -> p (s c)"),
                                        in1=RF0[:].broadcast_to((P, 2)), op=Alu.mult)
                nc.scalar.copy(out=F[:, 0:1], in_=F[:, T:T + 1])
                nc.scalar.copy(out=F[:, T + 1:T + 2], in_=F[:, 2 * T + 1:2 * T + 2])

    nc.compile()
    return nc


def _host_prep(u_r, u_i, x_r, x_i, w0_r, w0_i, f0_r, f0_i):
    l_total = u_r.shape[1]
    nblk = l_total // T
    urf = np.ascontiguousarray(u_r.reshape(B, l_total, KD // 2))
    uif = np.ascontiguousarray(u_i.reshape(B, l_total, KD // 2))
    ue = (u_r * u_r + u_i * u_i).sum(axis=(2, 3)) + EPS
    alpha = (LR_W / ue).astype(np.float32)

    tri = np.tril(np.ones((T, T), np.float32), -1).T
    trid = np.concatenate([tri, tri], axis=1)
    trisd = np.concatenate([-tri, tri], axis=1)

    wrf = np.swapaxes(w0_r, 2, 3).reshape(B, NM, KD // 2)
    wif = np.swapaxes(w0_i, 2, 3).reshape(B, NM, KD // 2)

    in_maps = []
    for c in range(NCORES):
        bs = slice(c * BC, (c + 1) * BC)
        urc = urf[bs].reshape(BC, nblk, T, KD // 2)
        uic = uif[bs].reshape(BC, nblk, T, KD // 2)
        alc = alpha[bs].reshape(BC, nblk, T).transpose(1, 0, 2)     # [nblk,BC,T]
        UA = np.concatenate([urc, uic], axis=3).transpose(1, 0, 3, 2)  # [nblk,BC,K,T]
        UB = np.concatenate([uic, -urc], axis=3).transpose(1, 0, 3, 2)
        rhsu = np.concatenate([UA, UB], axis=3).astype(np.float32)
        glhs = np.ascontiguousarray(UA * alc[:, :, None, :]).astype(np.float32)
        ua_t = UA.transpose(0, 1, 3, 2)
        ub_t = UB.transpose(0, 1, 3, 2)
        al_t = alc[:, :, :, None]
        u4 = np.concatenate([ua_t * al_t, ub_t * al_t], axis=2).astype(np.float32)
        xrc = x_r[bs].reshape(BC, nblk, T, NM)
        xic = x_i[bs].reshape(BC, nblk, T, NM)
        xpt = np.empty((nblk, P, 2, T2), np.float32)
        for b in range(BC):
            for i in range(NM):
                p = 2 * b + i
                xpt[:, p, 0, 0:T] = xrc[b, :, :, i]
                xpt[:, p, 0, T:T2] = xic[b, :, :, i]
                xpt[:, p, 1, 0:T] = xic[b, :, :, i]
                xpt[:, p, 1, T:T2] = -xrc[b, :, :, i]
        w0tt = np.empty((KD, P), np.float32)
        f0tt = np.empty((P, 2), np.float32)
        phi0 = np.empty((P, 2), np.float32)
        for b in range(BC):
            for i in range(NM):
                p = 2 * b + i
                w0tt[0:KD // 2, p] = wrf[c * BC + b, i]
                w0tt[KD // 2:, p] = -wif[c * BC + b, i]
                fr = float(f0_r[c * BC + b, i]); fi = float(f0_i[c * BC + b, i])
                f0tt[p, 0] = fr; f0tt[p, 1] = fi
                af = max(np.sqrt(fr * fr + fi * fi), 1e-30)
                phi0[p, 0] = fr / af; phi0[p, 1] = fi / af
        in_maps.append({
            "a_rhsu": np.ascontiguousarray(rhsu),
            "a_glhs": glhs,
            "a_u4": np.ascontiguousarray(u4),
            "a_xp": xpt,
            "a_w0t": w0tt, "a_f0t": f0tt, "a_phi0t": phi0,
            "a_tri": trid, "a_trisgn": trisd,
        })
    return in_maps


def kernel(u_r, u_i, x_r, x_i, w0_r, w0_i, f0_r, f0_i, _want_results=False,
           _trace=False):
    l_total = u_r.shape[1]
    nblk = l_total // T
    if nblk not in _BUILT:
        _BUILT[nblk] = _build_nc(nblk)
    nc = _BUILT[nblk]
    in_maps = _host_prep(u_r, u_i, x_r, x_i, w0_r, w0_i, f0_r, f0_i)
    kw = {"trace": True} if _trace else {}
    res = run_bass_kernel_spmd(nc, in_maps, core_ids=list(range(NCORES)), **kw)
    out = np.empty((B, l_total, NM, 2), np.float32)
    for c in range(NCORES):
        kov = res.results[c]["ko"].reshape(P, nblk, 2, T)
        for b in range(BC):
            for i in range(NM):
                p = 2 * b + i
                out[c * BC + b, :, i, 0] = kov[p, :, 0, :].reshape(l_total)
                out[c * BC + b, :, i, 1] = kov[p, :, 1, :].reshape(l_total)
    if _want_results:
        return out, res
    return out

